# revision 48
# baseline (speedup 1.0000x reference)
"""Trainium2 Bass kernel for masked causal multi-head attention.

Problem (hardcoded):
    x: (4, 2048, 512) f32, m: (4, 2048, 1) f32 (prefix 0/1 mask),
    w_qkv: (512, 1536) f32, w_out: (512, 512) f32, b_out: (512,) f32
    out = (softmax(mask(QK^T/8)) V) @ w_out + b_out, masked by m.

Sharding: 8 cores = 4 batches x 2 head-groups (4 heads each).  Each core
computes qkv projection for its (batch, head-group), flash-style causal
attention, and a partial out-projection; the host sums the two partials
per batch (replaces the all-reduce) and adds b_out.

Layout strategy (all compute bf16, accumulation f32 in PSUM):
  - x is passed pre-transposed per core: xt (512, L) where L = 128*nblk,
    nblk = ceil(maxlen/128) key/query blocks actually needed.
  - Weights arrive pre-packed in their SBUF layouts ([128, 1024] each)
    so each is a single DMA; the attention scale is folded into wq on
    the host.
  - Q^T, K^T computed in (dh, t) layout, two heads stacked per 128
    partitions -> scores are computed transposed: S^T (k, q) tiles, which
    makes softmax need no transposes anywhere:
      * no max-subtraction (scores are ~N(0,1.2), |s| <= ~9; exp is safe
        in f32/bf16) -> no running max, plain PSUM accumulation
      * exp on the scalar engine, causal triangle handled by multiplying
        a 0/1 triangle into the diagonal blocks after exp (split across
        the vector and gpsimd engines per head)
      * row sums l come from an extra all-ones column appended to V
      * O^T = V_aug^T P accumulated over key blocks in PSUM
  - 1/l for BOTH heads of a pair is broadcast across 128 partitions with
    a single tiny bf16 matmul (lhsT = 2x128 0/1 block indicator) and
    applied to ot in one multiply.  bf16 matmuls are 4x faster than the
    f32 ones the previous version used.
  - Out-projection contracts the two stacked heads per matmul, is masked
    by m per row during the PSUM->SBUF copy, and the whole superblock is
    written back with one DMA.
"""

import sys
from collections import deque

import numpy as np

try:
    import concourse.bass as bass  # noqa: F401
except ImportError:  # pragma: no cover
    sys.path.insert(0, "/opt/trn_rl_repo")

import concourse.bacc as bacc
import concourse.mybir as mybir
import concourse.tile as tile
from concourse import bass_utils

F32 = mybir.dt.float32
BF16 = mybir.dt.bfloat16
NP_BF16 = mybir.dt.np(BF16)
AF = mybir.ActivationFunctionType

B, T, D, H = 4, 2048, 512, 8
DH = D // H  # 64
G = 2  # head groups (cores per batch)
HPG = H // G  # heads per group = 4
SCALE = DH**-0.5
N_CORES = 8
N_WARMUP = 85  # PE clock-ramp matmuls covering the input-DMA lead-in


def build_nc(nblk: int):
    """Build the single SPMD Bass graph (same program on all 8 cores)."""
    L = nblk * 128
    NS = (L + 511) // 512  # number of 512-wide query superblocks

    def fs(s):  # query width of superblock s
        return min(512, L - 512 * s)

    def kbmax(s):  # causal+clamp bound on key blocks for superblock s
        return min(4 * s + (fs(s) + 127) // 128, nblk)

    nc = bacc.Bacc(
        "TRN2",
        target_bir_lowering=False,
        debug=False,
        enable_asserts=False,
        num_devices=N_CORES,
    )
    xt_d = nc.dram_tensor("xt", [D, L], BF16, kind="ExternalInput").ap()
    wq_d = nc.dram_tensor("wq", [128, 1024], BF16, kind="ExternalInput").ap()
    wk_d = nc.dram_tensor("wk", [128, 1024], BF16, kind="ExternalInput").ap()
    wv_d = nc.dram_tensor("wv", [128, 1024], BF16, kind="ExternalInput").ap()
    wo_d = nc.dram_tensor("wo", [128, 1024], BF16, kind="ExternalInput").ap()
    m_d = nc.dram_tensor("m", [128, nblk], F32, kind="ExternalInput").ap()
    tri_d = nc.dram_tensor("tri", [128, 128], BF16, kind="ExternalInput").ap()
    e_d = nc.dram_tensor("e", [65, 128], BF16, kind="ExternalInput").ap()
    out_d = nc.dram_tensor("out", [T, D], BF16, kind="ExternalOutput").ap()

    with tile.TileContext(nc) as tc:
        with (
            tc.tile_pool(name="const", bufs=1) as cpool,
            tc.tile_pool(name="work", bufs=3) as wpool,
            tc.tile_pool(name="ps", bufs=2, space="PSUM") as pspool,
            tc.tile_pool(name="pwork", bufs=5) as ppool,
            tc.tile_pool(name="s_ps", bufs=3, space="PSUM") as spool,
            tc.tile_pool(name="o_ps", bufs=3, space="PSUM") as opool,
        ):
            # ---- warm-up prep first so the PE can start ramping ASAP ----
            wu_sb = cpool.tile([128, 128], BF16, tag="wu", name="wu_sb")
            nc.gpsimd.memset(wu_sb[:], 0.0)

            # ---- persistent inputs -> SBUF ----
            # DMA priority == issue order: xt chunk 0 and wq/wk gate the
            # first qkv projection, so they go first; chunk 1 and the
            # late-use weights (wo, m, tri) trail.  Four engines' queues
            # spread the issue cost.
            dma_engines = [nc.sync, nc.scalar, nc.gpsimd, nc.sync]
            xt = [
                cpool.tile([128, L], BF16, tag=f"xt{d4}", name=f"xt{d4}")
                for d4 in range(4)
            ]

            def fetch_xt(s):
                if s >= NS:
                    return
                c0, w = 512 * s, fs(s)
                for d4 in range(4):
                    dma_engines[d4].dma_start(
                        xt[d4][:, c0 : c0 + w],
                        xt_d[128 * d4 : 128 * (d4 + 1), c0 : c0 + w],
                    )

            fetch_xt(0)
            wq_sb = cpool.tile([128, 1024], BF16, tag="wq", name="wq_sb")
            nc.sync.dma_start(wq_sb[:, 0:512], wq_d[:, 0:512])
            wk_sb = cpool.tile([128, 1024], BF16, tag="wk", name="wk_sb")
            nc.scalar.dma_start(wk_sb[:, 0:512], wk_d[:, 0:512])
            nc.sync.dma_start(wq_sb[:, 512:1024], wq_d[:, 512:1024])
            nc.scalar.dma_start(wk_sb[:, 512:1024], wk_d[:, 512:1024])
            wv_sb = cpool.tile([128, 1024], BF16, tag="wv", name="wv_sb")
            nc.gpsimd.dma_start(wv_sb[:], wv_d[:])
            # Only wo is throttled behind wq (1-column dummy copy, then the
            # DMA overwrites it): it is not needed until the first
            # out-projection, and keeping its 256KB out of the critical
            # chunk-0 + wq preload window starts the first matmul earlier.
            # Throttling xt chunk 1 the same way was tried and starves the
            # s=1 projections instead.
            wo_sb = cpool.tile([128, 1024], BF16, tag="wo", name="wo_sb")
            nc.vector.tensor_copy(wo_sb[:, 0:1], wq_sb[:, 0:1])
            fetch_xt(1)
            nc.sync.dma_start(wo_sb[:], wo_d[:])
            m_sb = cpool.tile([128, nblk], F32, tag="m", name="m_sb")
            nc.sync.dma_start(m_sb[:], m_d[:])
            tri_sb = cpool.tile([128, 128], BF16, tag="tri", name="tri_sb")
            nc.gpsimd.dma_start(tri_sb[:], tri_d[:])

            # Block indicator rows: row hi is 1 on partitions of head hi
            # (row 64 duplicates row 1 at a legal matmul base partition).
            e_sb = cpool.tile([65, 128], BF16, tag="e", name="e_sb")
            nc.gpsimd.dma_start(e_sb[:], e_d[:])

            # HAM warm-up: a dense burst of dummy full-array matmuls during
            # the DMA lead-in so the PE clock is ramped when real work
            # starts (the ramp needs ~3.4us of sustained activity).
            wu_ps = pspool.tile([128, 512], F32, tag="ps", name="wu_ps")
            for _ in range(N_WARMUP):
                nc.tensor.matmul(
                    wu_ps[:, :128], lhsT=wu_sb[:], rhs=wu_sb[:], start=True, stop=True
                )

            # ---- qkv projections, built per superblock (see below) ----
            # Q^T and K^T: (dh, t) with the pair's two heads stacked on
            # partitions; V: (k, dh) per key block, 4 heads side by side,
            # each with an extra all-ones 65th column (row-sum trick).
            qt = {}
            kt = [{}, {}]  # per-head K tiles, zero-padded to 128 rows so
            # the S matmul runs as a full 128x128-shaped matmul against the
            # stacked Q (the zero rows kill the other head's contribution)
            v = {}

            def qk_group(s, hp, which):
                def emit():
                    w = fs(s)
                    wsb = wq_sb if which == "q" else wk_sb
                    ps = pspool.tile([128, 512], F32, tag="ps", name="ps")
                    for d4 in range(4):
                        col = 128 * (4 * hp + d4)
                        nc.tensor.matmul(
                            ps[:, :w],
                            lhsT=wsb[:, col : col + 128],
                            rhs=xt[d4][:, 512 * s : 512 * s + w],
                            start=(d4 == 0),
                            stop=(d4 == 3),
                        )
                    if which == "q":
                        dst = cpool.tile([128, w], BF16, tag=f"qt{hp}_{s}", name=f"qt{hp}_{s}")
                        nc.vector.tensor_copy(dst[:], ps[:, :w])
                        qt[(hp, s)] = dst
                        return
                    for hi in range(2):
                        p0 = 64 * hi
                        dst = cpool.tile([128, w], BF16, tag=f"kt{hi}_{hp}_{s}", name=f"kt{hi}_{hp}_{s}")
                        nc.gpsimd.memset(dst[64 - p0 : 128 - p0, :], 0.0)
                        nc.vector.tensor_copy(
                            dst[p0 : p0 + 64, :], ps[p0 : p0 + 64, :w]
                        )
                        kt[hi][(hp, s)] = dst

                return emit

            def v_group(s, kb):
                def emit():
                    ps = pspool.tile([128, 512], F32, tag="ps", name="ps")
                    for d4 in range(4):
                        nc.tensor.matmul(
                            ps[:, :256],
                            lhsT=xt[d4][:, 128 * kb : 128 * (kb + 1)],
                            rhs=wv_sb[:, 256 * d4 : 256 * (d4 + 1)],
                            start=(d4 == 0),
                            stop=(d4 == 3),
                        )
                    vt = cpool.tile([128, HPG * 128], BF16, tag=f"v{kb}", name=f"v{kb}")
                    v3 = vt[:].rearrange("p (h c) -> p h c", c=128)
                    nc.gpsimd.memset(v3[:, :, 64:65], 1.0)
                    nc.gpsimd.memset(v3[:, :, 65:128], 0.0)
                    nc.vector.tensor_copy(
                        v3[:, :, 0:64], ps[:, :256].rearrange("p (h c) -> p h c", c=64)
                    )
                    v[kb] = vt

                return emit

            def qk_thunks(s):
                return [
                    qk_group(s, hp, which)
                    for hp in range(2)
                    for which in ("q", "k")
                ]

            def v_thunks(s):
                return [
                    v_group(s, kb)
                    for kb in range(4 * s, min(4 * s + (fs(s) + 127) // 128, nblk))
                ]

            # ---- attention + out-projection ----
            # Heads are processed in (h0, h1) pairs with a one-chunk
            # software pipeline (emit S(c) for both heads, then exp(c),
            # then AV(c-1)) so the exp latency is hidden behind the other
            # head's matmuls.  Each pair's normalize chain starts right at
            # pair end (frees the O PSUM slots fast), but the PE-side
            # broadcast matmul + final multiply are deferred until after
            # the next pair; each superblock's out-projection is deferred
            # into the next superblock.  The PE stream therefore never
            # waits on the reciprocal chain.
            def plan_chunks(s):
                F = fs(s)
                KB = kbmax(s)
                chunks = []
                segs = []
                used = 0
                for kb in range(KB):
                    qoff = max(0, 128 * (kb - 4 * s))
                    feff = F - qoff
                    off = used
                    if off // 512 != (off + feff - 1) // 512:
                        off = 512 * ((off + 511) // 512)  # next bank
                    if off + feff > 512:
                        chunks.append((segs, used))
                        segs = []
                        off = 0
                    segs.append((kb, qoff, feff, off))
                    used = off + feff
                if segs:
                    chunks.append((segs, used))
                return chunks

            def attention_pair(s, hp, o_ps2, ot_sb, filler=None, post=None):
                F = fs(s)
                KB = kbmax(s)
                chunks = plan_chunks(s)
                done = [0, 0]

                def s_matmul(hi, out_ap, kb, qoff, feff):
                    tck, off = divmod(kb, 4)
                    nc.tensor.matmul(
                        out_ap,
                        lhsT=kt[hi][(hp, tck)][:, 128 * off : 128 * off + 128],
                        rhs=qt[(hp, s)][:, qoff : qoff + feff],
                        start=True,
                        stop=True,
                    )

                def do_avs(hi, segs, p_sb):
                    h = 2 * hp + hi
                    for kb, qoff, feff, off in segs:
                        nc.tensor.matmul(
                            o_ps2[hi][0:128, qoff : qoff + feff],
                            lhsT=v[kb][:, 128 * h : 128 * h + 128],
                            rhs=p_sb[:, off : off + feff],
                            start=(done[hi] == 0),
                            stop=(done[hi] == KB - 1),
                        )
                        done[hi] += 1

                prev = None  # (segs, [p_sb x2])
                for segs, used in chunks:
                    ps2 = []
                    for hi in range(2):
                        s_ps = spool.tile([128, 512], F32, tag="s", name="s_ps")
                        for kb, qoff, feff, off in segs:
                            s_matmul(hi, s_ps[:, off : off + feff], kb, qoff, feff)
                        ps2.append(s_ps)
                    if filler:
                        filler.popleft()()  # fill the exp bubble with next
                        # superblock's qkv projection work
                    pb2 = []
                    for hi in range(2):
                        p_sb = ppool.tile([128, 512], BF16, tag="p", name="p_sb")
                        nc.scalar.activation(p_sb[:, :used], ps2[hi][:, :used], AF.Exp)
                        tri_engine = nc.vector if hi == 0 else nc.gpsimd
                        for kb, qoff, feff, off in segs:
                            if kb >= 4 * s:
                                # diagonal: multiplicative causal triangle
                                tri_engine.tensor_mul(
                                    p_sb[:, off : off + 128],
                                    p_sb[:, off : off + 128],
                                    tri_sb[:],
                                )
                        pb2.append(p_sb)
                    if post is not None:
                        post()  # prev pair's deferred drains, now that this
                        post = None  # pair's first exps are queued
                    if prev is not None:
                        for hi in range(2):
                            do_avs(hi, prev[0], prev[1][hi])
                    prev = (segs, pb2)
                for hi in range(2):
                    do_avs(hi, prev[0], prev[1][hi])

            def start_normalize(s, hp, o_ps2, ot_sb):
                """Fast part: drain o_ps and start the reciprocal chain.
                Returns the deferred finisher (PE broadcast + multiply)."""
                F = fs(s)
                nq = (F + 127) // 128
                # first/last superblocks: fin chains have the least PE work
                # to hide behind, so use the DMA-free in-row approximate
                # reciprocal there; elsewhere the spread/gather DMA trick
                # keeps the (slow) in-row work off the vector engine.
                local = s == 0 or s == NS - 1
                rc2 = None
                if not local:
                    rc2 = wpool.tile([2, 512], BF16, tag="rc2", name="rc2")
                rbs = []
                lcols = []
                # boundary-immediate part (2 vector ops only): head-0 l row
                # and the head-1 O drain.  Everything scalar-side is deferred
                # past the next pair's first exps (see post below) so the
                # exp->AV chain at the boundary is never delayed.
                lrow0 = wpool.tile([1, 512], F32, tag="lrow", name="lrow")
                nc.vector.tensor_copy(lrow0[0:1, :F], o_ps2[0][64:65, :F])
                nc.vector.tensor_copy(ot_sb[hp][64:128, 0:F], o_ps2[1][0:64, :F])
                if local:
                    rrow = wpool.tile([1, 512], F32, tag="rrow", name="rrow")
                    nc.vector.reciprocal_approx_fast(rrow[0:1, :F], lrow0[0:1, :F])
                    rb = wpool.tile([65, 512], BF16, tag="rb0", name="rb0")
                    nc.vector.tensor_copy(rb[0:1, :F], rrow[0:1, :F])
                    rbs.append(rb)
                else:
                    lcol = wpool.tile([128, 4], F32, tag="lcol", name="lcol")
                    nc.gpsimd.dma_start(
                        lcol[:, 0:nq],
                        lrow0[0:1, :F].rearrange("o (p c) -> o p c", c=nq),
                    )
                    lcols.append(lcol)

                def post():
                    lrow1 = wpool.tile([1, 512], F32, tag="lrow", name="lrow")
                    nc.scalar.activation(
                        lrow1[0:1, :F], o_ps2[1][64:65, :F], AF.Copy
                    )
                    nc.scalar.activation(
                        ot_sb[hp][0:64, 0:F], o_ps2[0][0:64, :F], AF.Copy
                    )
                    if local:
                        rrow1 = wpool.tile([1, 512], F32, tag="rrow", name="rrow")
                        nc.vector.reciprocal_approx_fast(
                            rrow1[0:1, :F], lrow1[0:1, :F]
                        )
                        rb1 = wpool.tile([65, 512], BF16, tag="rb1", name="rb1")
                        nc.vector.tensor_copy(rb1[64:65, :F], rrow1[0:1, :F])
                        rbs.append(rb1)
                        return
                    lcol1 = wpool.tile([128, 4], F32, tag="lcol", name="lcol")
                    nc.gpsimd.dma_start(
                        lcol1[:, 0:nq],
                        lrow1[0:1, :F].rearrange("o (p c) -> o p c", c=nq),
                    )
                    lcols.append(lcol1)
                    for hi in range(2):
                        rcol = wpool.tile([128, 4], F32, tag="rcol", name="rcol")
                        nc.vector.reciprocal(rcol[:, 0:nq], lcols[hi][:, 0:nq])
                        rbcol = wpool.tile([128, 4], BF16, tag="rbcol", name="rbcol")
                        nc.vector.tensor_copy(rbcol[:, 0:nq], rcol[:, 0:nq])
                        nc.gpsimd.dma_start(
                            rc2[hi : hi + 1, :F].rearrange("o (p c) -> o p c", c=nq),
                            rbcol[:, 0:nq],
                        )

                def finish():
                    bc = pspool.tile([128, 512], F32, tag="ps", name="bc_ps")
                    if local:
                        # two independent matmuls into disjoint partition
                        # ranges (no cross-base accumulation group)
                        for hi in range(2):
                            r0 = 64 * hi
                            nc.tensor.matmul(
                                bc[r0 : r0 + 64, :F],
                                lhsT=e_sb[r0 : r0 + 1, r0 : r0 + 64],
                                rhs=rbs[hi][r0 : r0 + 1, :F],
                                start=True,
                                stop=True,
                            )
                    else:
                        nc.tensor.matmul(
                            bc[0:128, :F],
                            lhsT=e_sb[0:2, :],
                            rhs=rc2[0:2, :F],
                            start=True,
                            stop=True,
                        )
                    nc.vector.tensor_mul(
                        ot_sb[hp][:, 0:F], ot_sb[hp][:, 0:F], bc[0:128, :F]
                    )

                return post, finish

            def make_outproj(s, ot_sb):
                def outproj():
                    F = fs(s)
                    nq = (F + 127) // 128
                    ob = wpool.tile([128, 2048], BF16, tag="ob", name="ob")
                    for qi in range(nq):
                        y_ps = pspool.tile([128, 512], F32, tag="ps", name="ps")
                        for hp in range(2):
                            nc.tensor.matmul(
                                y_ps[:],
                                lhsT=ot_sb[hp][:, 128 * qi : 128 * (qi + 1)],
                                rhs=wo_sb[:, 512 * hp : 512 * (hp + 1)],
                                start=(hp == 0),
                                stop=(hp == 1),
                            )
                        qg = 4 * s + qi
                        obr = ob[:, 512 * qi : 512 * (qi + 1)]
                        if s == NS - 1:
                            # tail: scalar is idle once the exps are done
                            nc.scalar.activation(
                                obr, y_ps[:], AF.Copy, scale=m_sb[:, qg : qg + 1]
                            )
                        else:
                            nc.vector.tensor_scalar_mul(
                                obr, y_ps[:], m_sb[:, qg : qg + 1]
                            )
                    row = 512 * s
                    nc.sync.dma_start(
                        out_d[row : row + F, :].rearrange("(q p) c -> p q c", p=128),
                        ob[:, 0 : 512 * nq].rearrange("p (q c) -> p q c", c=512),
                    )

                return outproj

            pending_fin = None
            pending_post = None
            pending_out = None
            nexts = deque()
            for s in range(NS):
                for t in nexts:  # leftover projection work for this s
                    t()
                nexts = deque()
                fetch_xt(s + 2)
                if s == 0:
                    for t in qk_thunks(0) + v_thunks(0):
                        t()
                ot_sb = [
                    wpool.tile([128, 512], BF16, tag=f"ot{hp}", name=f"ot{hp}")
                    for hp in range(2)
                ]
                for hp in range(2):
                    o_ps2 = [
                        opool.tile([128, 512], F32, tag="o", name="o_ps")
                        for _ in range(2)
                    ]
                    filler = None
                    if s + 1 < NS:
                        if hp == 0:
                            nexts = deque(v_thunks(s + 1))
                        else:
                            nexts = deque(qk_thunks(s + 1)) + nexts
                        filler = nexts
                    attention_pair(s, hp, o_ps2, ot_sb, filler, post=pending_post)
                    pending_post = None
                    if pending_fin is not None:
                        pending_fin()
                    pending_post, pending_fin = start_normalize(s, hp, o_ps2, ot_sb)
                    if s == NS - 1 and hp == 1:
                        # no next pair whose exps the drains could delay --
                        # run them now so the tail fin chain isn't serial
                        pending_post()
                        pending_post = None
                    if hp == 0 and pending_out is not None:
                        pending_out()
                        pending_out = None
                pending_out = make_outproj(s, ot_sb)
            if pending_post is not None:
                pending_post()
            pending_fin()
            pending_out()

    nc.compile()
    return nc


def make_in_maps(x, m, w_qkv, w_out, nblk: int):
    """Host-side sharding/packing: core c = (batch c//2, head-group c%2).

    Weights are packed into their exact SBUF layouts so each is one DMA.
    """
    L = nblk * 128
    tri = np.where(
        np.arange(128)[None, :] >= np.arange(128)[:, None], 1.0, 0.0
    ).astype(NP_BF16)
    e = np.zeros((65, 128), NP_BF16)
    e[0, 0:64] = 1.0
    e[1, 64:128] = 1.0
    e[64, 64:128] = 1.0
    in_maps = []
    for c in range(N_CORES):
        b, g = divmod(c, 2)
        xt = np.ascontiguousarray(x[b].T[:, :L]).astype(NP_BF16)
        wq = np.empty((128, 1024), NP_BF16)
        wk = np.empty((128, 1024), NP_BF16)
        for hp in range(2):
            for d4 in range(4):
                col = 128 * (4 * hp + d4)
                rows = slice(128 * d4, 128 * (d4 + 1))
                qcol = 256 * g + 128 * hp
                wq[:, col : col + 128] = (w_qkv[rows, qcol : qcol + 128] * SCALE).astype(NP_BF16)
                wk[:, col : col + 128] = w_qkv[rows, 512 + qcol : 512 + qcol + 128].astype(NP_BF16)
        wv = np.empty((128, 1024), NP_BF16)
        for d4 in range(4):
            wv[:, 256 * d4 : 256 * (d4 + 1)] = w_qkv[
                128 * d4 : 128 * (d4 + 1), 1024 + 256 * g : 1024 + 256 * (g + 1)
            ].astype(NP_BF16)
        wo = np.empty((128, 1024), NP_BF16)
        for hp in range(2):
            r0 = 256 * g + 128 * hp
            wo[:, 512 * hp : 512 * (hp + 1)] = w_out[r0 : r0 + 128, :].astype(NP_BF16)
        mp = np.ascontiguousarray(
            m[b, :L, 0].reshape(nblk, 128).T
        ).astype(np.float32)
        in_maps.append(
            {"xt": xt, "wq": wq, "wk": wk, "wv": wv, "wo": wo, "m": mp, "tri": tri, "e": e}
        )
    return in_maps


def postprocess(results, x, m, b_out):
    out = np.zeros((B, T, D), np.float32)
    for b in range(B):
        out[b] = results[2 * b]["out"].astype(np.float32) + results[
            2 * b + 1
        ]["out"].astype(np.float32)
    out += b_out[None, None, :].astype(np.float32) * m.astype(np.float32)
    return out


def kernel(x, m, w_qkv, w_out, b_out):
    lengths = m[:, :, 0].astype(np.int64).sum(axis=1)
    nblk = max(1, int(-(-lengths.max() // 128)))
    nc = build_nc(nblk)
    in_maps = make_in_maps(x, m, w_qkv, w_out, nblk)
    res = bass_utils.run_bass_kernel_spmd(nc, in_maps, core_ids=list(range(N_CORES)))
    return postprocess(res.results, x, m, b_out)
